# revision 26
# baseline (speedup 1.0000x reference)
"""Trainium2 kernel for the modality-softmax attention problem.

    scores  = tanh(einsum("mbd,ed->mbe", x, W))
    weights = softmax(scores, axis=0)            # over M modalities
    out     = sum_m x * weights                  # [B, D]
    out    *= (1 + #modalities whose feature-sum is exactly 0)[b]

Sharding: data-parallel over the batch dim — 8 NeuronCores x B/8 rows,
W replicated. Everything on-chip runs transposed ([feature, batch]).

The score matmuls (the FLOP bottleneck: 2*M*B*D^2 = 412 GFLOP) run in
fp8e4 with perf_mode=DoubleRow: each matmul contracts K=256 (two 128-row
sub-chunks packed per PE cell) in the same ~N cycles a bf16 K=128 matmul
takes — a true 2x (measured 216 ns per K=256,N=512 matmul, DR ldweights
fully hidden). W is scaled by 16 before fp8 quantization (its entries
~N(0, 1/D) would land in the subnormal range) and the 1/16 is folded into
the tanh activation's scale. The elementwise path runs in fp16 to keep
quantization error concentrated in the fp8 matmul: host ships two copies
of x — fp8 for the matmul moving operand, fp16 for the elementwise
operand (d-major layout serves both roles since D == E).

Engine budget per core: Tensor ~311 us is the roofline. The softmax
accumulation (x*e product, num add, den add) is the next-largest load;
DVE tensor_tensor slows 2-3x under SBUF contention while the matmul
stream runs, so ops are pair-fused (two 128-row e-chunks per [P,2,BT]
instruction, one 2-bank PSUM tile per tanh) and split between DVE and
the Pool engine (stable ~2.4 us/pair-op). The per-pair finalize
(den->f32 copy on ACT, fast reciprocal, num*recip mul, store) is
deferred OUT of the contended stream: b-tile k's finalizes run during
b-tile k+1's first modalities (where DVE is idle), and the last b-tile's
trail after the matmul stream, output muls alternating DVE/Pool. The
missing-modality rescale is applied on the host during the gather (the
host knows the zero rows exactly; straggler zero rows inside the kernel
still contribute exp(0)=1 to Den and 0 to Num naturally, so on-device
detection is unnecessary).

tanh scores lie in [-1,1], so softmax is computed without max-subtraction
as (sum x*exp(tanh s)) / (sum exp(tanh s)).

Missing-modality rows (all-zero x[m, b, :]) contribute exp(0)=1 to the
softmax denominator and 0 to the numerator, so their matmuls are pure
waste. The host detects them, permutes the batch so each core sees the
same per-modality all-zero prefix, and the kernel is built with those
prefixes statically skipped (shorter matmul N, Den bumped by a constant).
The permutation is undone on the host after the gather. With no zero rows
the plan degenerates to a dense kernel.

DMA: wt streams on the ACT ring (idle early), x8 on sync, xe on Pool,
outputs on sync; DRAM layouts are partition-major so every transfer
reads >=2KB contiguous per partition. x tiles are DMA-issued one
work-item ahead; Tile's WAR semaphores start each transfer the moment
its double-buffer slot frees.
"""

from contextlib import ExitStack

import numpy as np
import ml_dtypes

import concourse.bass as bass
import concourse.bacc as bacc
import concourse.mybir as mybir
import concourse.tile as tile
from concourse.bass_utils import run_bass_kernel_spmd

F32 = mybir.dt.float32
FP16 = mybir.dt.float16
FP8 = mybir.dt.float8e4
P = 128
N_CORES = 8
W_SCALE = 16.0  # pow2: exact in fp; folded into tanh via activation scale
GP_DEN_MODS = (1, 2, 3, 4, 5)  # modalities whose den-add runs on Pool


def build_kernel(M, D, E, Bc, BT, skips=None):
    """Build the per-core Bass graph.

    M: modalities, D: contraction dim, E: output feature dim, Bc: per-core
    batch, BT: batch tile (matmul N). skips[m] = per-core all-zero prefix
    length for modality m (columns statically skipped).
    """
    DC = D // P
    D2 = DC // 2  # DoubleRow chunks of K=256
    EC = E // P
    EP = EC // 2  # e-chunk pairs for per-op fusion
    NBT = Bc // BT
    assert D % (2 * P) == 0 and E % (2 * P) == 0 and Bc % BT == 0
    skips = list(skips or [0] * M)
    assert len(skips) == M and all(0 <= k <= Bc for k in skips)

    nc = bacc.Bacc()

    x8 = nc.declare_dram_parameter("x8", [M, NBT, P, DC, BT], FP8, isOutput=False)
    xe = nc.declare_dram_parameter("xe", [M, NBT, P, DC, BT], FP16, isOutput=False)
    wt = nc.declare_dram_parameter("wt", [EC, P, D], FP8, isOutput=False)
    outT = nc.declare_dram_parameter("outT", [E, Bc], F32, isOutput=True)

    with tile.TileContext(nc) as tc, ExitStack() as ctx:
        singles = ctx.enter_context(tc.tile_pool(name="singles", bufs=1))
        xe_pool = ctx.enter_context(tc.tile_pool(name="xe", bufs=2))
        x8_pool = ctx.enter_context(tc.tile_pool(name="x8", bufs=2))
        acc_pool = ctx.enter_context(tc.tile_pool(name="acc", bufs=2))
        e_pool = ctx.enter_context(tc.tile_pool(name="e", bufs=3))
        t_pool = ctx.enter_context(tc.tile_pool(name="t", bufs=2))
        prod_pool = ctx.enter_context(tc.tile_pool(name="prod", bufs=2))
        out_pool = ctx.enter_context(tc.tile_pool(name="out", bufs=3))
        rec_pool = ctx.enter_context(tc.tile_pool(name="rec", bufs=2))
        den_pool = ctx.enter_context(tc.tile_pool(name="den32", bufs=2))
        sc_psum = ctx.enter_context(tc.tile_pool(name="scps", bufs=3, space="PSUM"))
        wu_psum = ctx.enter_context(tc.tile_pool(name="wups", bufs=1, space="PSUM"))

        wu_sb = singles.tile([P, 64], mybir.dt.bfloat16)
        nc.vector.memset(wu_sb, 1.0)
        wu_ps = wu_psum.tile([P, 64], F32)
        for i in range(64):
            nc.tensor.matmul(
                wu_ps[:64], lhsT=wu_sb[:, :64], rhs=wu_sb,
                start=(i == 0), stop=(i == 63),
            )
        wu_out = singles.tile([P, 64], F32)
        nc.vector.tensor_copy(wu_out[:64], wu_ps[:64])

        tiles = {}

        def issue_loads(it):
            bt, m = it
            lo = min(max(skips[m] - bt * BT, 0), BT)
            xe_t = xe_pool.tile([P, DC, BT], FP16)
            x8_t = x8_pool.tile([P, DC, BT], FP8)
            for q in range(0, DC, max(DC // 4, 1)):
                qe = min(q + max(DC // 4, 1), DC)
                nc.sync.dma_start(
                    out=x8_t[:, q:qe, lo:], in_=x8[m, bt, :, q:qe, lo:]
                )
                nc.gpsimd.dma_start(
                    out=xe_t[:, q:qe, lo:], in_=xe[m, bt, :, q:qe, lo:]
                )
            tiles[it] = (xe_t, x8_t)

        def bt_info(bt):
            sk = [min(max(skips[mm] - bt * BT, 0), BT) for mm in range(M)]
            wm = [BT] * (M + 1)
            for mm in range(M):
                wm[mm + 1] = min(wm[mm], sk[mm])
            return sk, wm

        load_seq = []
        for bt in range(NBT):
            sk, _ = bt_info(bt)
            load_seq.extend(
                (bt, mm)
                for mm in sorted(
                    (m for m in range(M) if sk[m] < BT), key=lambda m: sk[m]
                )
            )

        # Replicated fp8 weight, partition-major DRAM layout (2KB contiguous
        # per partition per e-chunk), streamed on the sync ring interleaved
        # with consumption: two e-chunk pairs up front, the rest issued two
        # pairs ahead of the first modality's matmul pairs.
        wt_sb = singles.tile([P, EC, DC, P], FP8)
        wt_loaded = [0]

        def issue_wt_pair(wp):
            if wp < wt_loaded[0] or wp >= EC // 2:
                return
            for ec in (2 * wp, 2 * wp + 1):
                nc.sync.dma_start(out=wt_sb[:, ec, :, :], in_=wt[ec, :, :])
            wt_loaded[0] = wp + 1

        bt0, m0 = load_seq[0]
        lo0 = min(max(skips[m0] - bt0 * BT, 0), BT)
        xe_t = xe_pool.tile([P, DC, BT], FP16)
        x8_t = x8_pool.tile([P, DC, BT], FP8)
        for q in range(0, DC, max(DC // 4, 1)):
            qe = min(q + max(DC // 4, 1), DC)
            nc.sync.dma_start(
                out=wt_sb[:, 0, q:qe, :], in_=wt[0, :, q * P : qe * P]
            )
            nc.sync.dma_start(
                out=x8_t[:, q:qe, lo0:], in_=x8[m0, bt0, :, q:qe, lo0:]
            )
            nc.gpsimd.dma_start(
                out=xe_t[:, q:qe, lo0:], in_=xe[m0, bt0, :, q:qe, lo0:]
            )
        tiles[load_seq[0]] = (xe_t, x8_t)
        nc.sync.dma_start(out=wt_sb[:, 1, :, :], in_=wt[1, :, :])
        wt_loaded[0] = 1
        issue_wt_pair(1)

        loads_done = 1

        def issue_next_load():
            nonlocal loads_done
            if loads_done < len(load_seq):
                issue_loads(load_seq[loads_done])
                loads_done += 1

        # deferred finalizes: list of closures, popped during the next
        # b-tile's early (DVE-idle) windows
        pending_fin = []

        def make_finalize(bt, ep, n_sb, d_sb, sk, w_final):
            def run(split=1, alt=0):
                ec0 = 2 * ep
                ecs = slice(ec0, ec0 + 2)
                # statically-skipped modalities contribute exp(0)=1 to Den
                if w_final > 0:
                    nc.vector.memset(n_sb[:, ecs, :w_final], 0.0)
                    nc.vector.memset(d_sb[:, ecs, :w_final], 0.0)
                for mm in range(M):
                    if sk[mm] > 0:
                        nc.vector.tensor_scalar_add(
                            d_sb[:, ecs, : sk[mm]], d_sb[:, ecs, : sk[mm]], 1.0
                        )
                # fp16 den -> f32 for the bit-trick reciprocal; on the ACT
                # queue, which has headroom
                d32 = den_pool.tile([P, 2, BT], F32, tag="d32")
                nc.scalar.activation(
                    d32, d_sb[:, ecs, :], mybir.ActivationFunctionType.Copy
                )
                H = BT // split
                for h in range(split):
                    hs = slice(h * H, (h + 1) * H)
                    r_t = rec_pool.tile([P, 2, BT], F32, tag="recip")
                    # Den in [1, M*e]: no edge cases; 51-ULP approx is plenty
                    nc.vector.reciprocal_approx_fast(
                        out=r_t[:, :, hs], in_=d32[:, :, hs]
                    )
                    o_t = out_pool.tile([P, 2, BT], F32)
                    mul_eng = nc.gpsimd if alt else nc.vector
                    mul_eng.tensor_mul(
                        o_t[:, :, hs], n_sb[:, ecs, hs], r_t[:, :, hs]
                    )
                    nc.sync.dma_start(
                        out=outT[
                            ec0 * P : (ec0 + 2) * P,
                            bt * BT + h * H : bt * BT + (h + 1) * H,
                        ].rearrange("(c p) b -> p c b", c=2),
                        in_=o_t[:, :, hs],
                    )

            return run

        for bt in range(NBT):
            sk, wm = bt_info(bt)
            w_final = wm[M]
            mods = sorted(
                (mm for mm in range(M) if sk[mm] < BT), key=lambda mm: sk[mm]
            )
            n_sb = acc_pool.tile([P, EC, BT], FP16, tag="num")
            d_sb = acc_pool.tile([P, EC, BT], FP16, tag="den")

            # init watermark in PROCESSING order: columns < watermark are
            # still uninitialized when modality mods[i] runs
            watermark = BT
            for mi, m in enumerate(mods):
                lo = sk[m]  # live range [lo, BT)
                ini_hi = watermark  # init [lo, ini_hi), accum [ini_hi, BT)
                watermark = min(watermark, lo)
                xe_t, x8_t = tiles.pop((bt, m))

                for ep in range(EP):
                    if bt == 0 and mi == 0:
                        issue_wt_pair(ep + 2)
                    sc_ps = sc_psum.tile([P, 2, BT], F32)
                    for j in range(2):
                        ec = 2 * ep + j
                        for d2 in range(D2):
                            nc.tensor.matmul(
                                sc_ps[:, j, lo:],
                                lhsT=wt_sb[:, ec, 2 * d2 : 2 * d2 + 2, :],
                                rhs=x8_t[:, 2 * d2 : 2 * d2 + 2, lo:],
                                start=(d2 == 0),
                                stop=(d2 == D2 - 1),
                                perf_mode=mybir.MatmulPerfMode.DoubleRow,
                            )
                    # pair-fused activations: tanh over both psum banks at once
                    t2 = t_pool.tile([P, 2, BT], FP16, tag="tanh")
                    nc.scalar.activation(
                        t2[:, :, lo:], sc_ps[:, :, lo:],
                        mybir.ActivationFunctionType.Tanh,
                        scale=1.0 / W_SCALE,
                    )
                    ecs = slice(2 * ep, 2 * ep + 2)
                    e2 = None
                    if ini_hi > lo:
                        nc.scalar.activation(
                            d_sb[:, ecs, lo:ini_hi],
                            t2[:, :, lo:ini_hi],
                            mybir.ActivationFunctionType.Exp,
                        )
                    acc_lo = max(lo, ini_hi)
                    if BT > acc_lo:
                        e2 = e_pool.tile([P, 2, BT], FP16, tag="exp")
                        nc.scalar.activation(
                            e2[:, :, acc_lo:],
                            t2[:, :, acc_lo:],
                            mybir.ActivationFunctionType.Exp,
                        )

                    # paired accumulation over both e-chunks at once
                    if ini_hi > lo:
                        ini_eng = nc.gpsimd if ini_hi == BT else nc.vector
                        ini_eng.tensor_mul(
                            n_sb[:, ecs, lo:ini_hi],
                            xe_t[:, ecs, lo:ini_hi],
                            d_sb[:, ecs, lo:ini_hi],
                        )
                    if BT > acc_lo:
                        last_item = bt == NBT - 1 and m == mods[-1]
                        p2 = prod_pool.tile([P, 2, BT], FP16, tag="prod")
                        mul_eng = nc.gpsimd if last_item else nc.vector
                        mul_eng.tensor_mul(
                            p2[:, :, acc_lo:], xe_t[:, ecs, acc_lo:], e2[:, :, acc_lo:]
                        )
                        nc.vector.tensor_add(
                            n_sb[:, ecs, acc_lo:], n_sb[:, ecs, acc_lo:],
                            p2[:, :, acc_lo:],
                        )
                        den_eng = (
                            nc.gpsimd
                            if (m in GP_DEN_MODS and not last_item)
                            else nc.vector
                        )
                        den_eng.tensor_add(
                            d_sb[:, ecs, acc_lo:], d_sb[:, ecs, acc_lo:],
                            e2[:, :, acc_lo:],
                        )

                    # previous b-tile's finalizes run here, where DVE is idle
                    if pending_fin and mi < 3:
                        pending_fin.pop(0)(alt=(mi + ep) % 2)

                # next x tiles issue after this item's wt prefetches on sync
                issue_next_load()

            assert not pending_fin
            for ep in range(EP):
                pending_fin.append(make_finalize(bt, ep, n_sb, d_sb, sk, w_final))

        # last b-tile's finalizes trail the matmul stream (uncontended ops);
        # alternate the output mul between DVE and Pool
        for i, fin in enumerate(pending_fin):
            fin(split=4 if i == len(pending_fin) - 1 else 1, alt=i % 2)
        pending_fin.clear()

    nc.compile()
    return nc


def plan_shards(x):
    """Assign batch rows to cores so every core sees the same per-modality
    all-zero prefix. Returns (perm [n_cores, Bc] of global row ids,
    skips [M] prefix lengths)."""
    M, B, D = x.shape
    Bc = B // N_CORES
    zero = np.all(x == 0.0, axis=2)  # [M, B] truly-all-zero rows
    zcount = zero.sum(axis=0)
    sig = np.zeros(B, dtype=np.int64)
    for m in range(M):
        sig |= zero[m].astype(np.int64) << m
    # cluster by signature with most-zero rows first; round-robin over cores
    order = np.lexsort((sig, -zcount))
    perm = np.stack([order[c::N_CORES] for c in range(N_CORES)])
    skips = []
    for m in range(M):
        k = Bc
        for c in range(N_CORES):
            nz = np.flatnonzero(~zero[m][perm[c]])
            k = min(k, int(nz[0]) if len(nz) else Bc)
        skips.append(k)
    return perm, skips


def prepare_inputs(x, W, perm, BT):
    """Host-side packing: per-core permuted shard, partition-major layouts;
    fp8 copy for the matmul path, fp16 copy for the elementwise path."""
    M, B, D = x.shape
    Bc = B // N_CORES
    DC = D // P
    EC = D // P
    NBT = Bc // BT
    # wt[ec, p, dc, j] = (W.T * S)[dc*P + p, ec*P + j]
    wt = np.ascontiguousarray(
        (W.T * W_SCALE)
        .astype(ml_dtypes.float8_e4m3)
        .reshape(DC, P, EC, P)
        .transpose(2, 1, 0, 3)
    ).reshape(EC, P, DC * P)
    x8b = x.astype(ml_dtypes.float8_e4m3)
    xeb = x.astype(np.float16)
    in_maps = []
    for c in range(N_CORES):
        # [M, NBT, P, DC, BT] with value (m, bt, p, dc, b) =
        #   x[m, perm[bt*BT+b], dc*P+p]
        x8c = np.ascontiguousarray(
            x8b[:, perm[c], :].reshape(M, NBT, BT, DC, P).transpose(0, 1, 4, 3, 2)
        )
        xec = np.ascontiguousarray(
            xeb[:, perm[c], :].reshape(M, NBT, BT, DC, P).transpose(0, 1, 4, 3, 2)
        )
        in_maps.append({"x8": x8c, "xe": xec, "wt": wt})
    return in_maps


_NC_CACHE = {}


def kernel(x, W, _trace=False, **trace_kwargs):
    x = np.asarray(x)
    W = np.asarray(W)
    M, B, D = x.shape
    Bc = B // N_CORES
    BT = 512 if Bc % 512 == 0 else Bc
    perm, skips = plan_shards(x)
    key = (M, B, D, tuple(skips))
    if key not in _NC_CACHE:
        _NC_CACHE[key] = build_kernel(M, D, D, Bc, BT, skips=skips)
    nc = _NC_CACHE[key]
    in_maps = prepare_inputs(x, W, perm, BT)
    res = run_bass_kernel_spmd(
        nc, in_maps, core_ids=list(range(N_CORES)), trace=_trace, **trace_kwargs
    )
    # missing-modality rescale on the host (reference-exact semantics:
    # a modality counts as missing when its feature row sums to 0)
    scaler = ((x.sum(axis=-1) == 0.0).sum(axis=0) + 1).astype(np.float32)
    out = np.empty((B, D), np.float32)
    for c in range(N_CORES):
        out[perm[c], :] = (
            res.results[c]["outT"]
            * scaler[perm[c]][None, :]
        ).T
    if _trace:
        return out, res
    return out


# revision 27
# speedup vs baseline: 1.0097x; 1.0097x over previous
"""Trainium2 kernel for the modality-softmax attention problem.

    scores  = tanh(einsum("mbd,ed->mbe", x, W))
    weights = softmax(scores, axis=0)            # over M modalities
    out     = sum_m x * weights                  # [B, D]
    out    *= (1 + #modalities whose feature-sum is exactly 0)[b]

Sharding: data-parallel over the batch dim — 8 NeuronCores x B/8 rows,
W replicated. Everything on-chip runs transposed ([feature, batch]).

The score matmuls (the FLOP bottleneck: 2*M*B*D^2 = 412 GFLOP) run in
fp8e4 with perf_mode=DoubleRow: each matmul contracts K=256 (two 128-row
sub-chunks packed per PE cell) in the same ~N cycles a bf16 K=128 matmul
takes — a true 2x (measured 216 ns per K=256,N=512 matmul, DR ldweights
fully hidden). W is scaled by 16 before fp8 quantization (its entries
~N(0, 1/D) would land in the subnormal range) and the 1/16 is folded into
the tanh activation's scale. The elementwise path runs in fp16 to keep
quantization error concentrated in the fp8 matmul: host ships two copies
of x — fp8 for the matmul moving operand, fp16 for the elementwise
operand (d-major layout serves both roles since D == E).

Engine budget per core: Tensor ~311 us is the roofline. The softmax
accumulation (x*e product, num add, den add) is the next-largest load;
DVE tensor_tensor slows 2-3x under SBUF contention while the matmul
stream runs, so ops are pair-fused (two 128-row e-chunks per [P,2,BT]
instruction, one 2-bank PSUM tile per tanh) and split between DVE and
the Pool engine (stable ~2.4 us/pair-op). The per-pair finalize
(den->f32 copy on ACT, fast reciprocal, num*recip mul, store) is
deferred OUT of the contended stream: b-tile k's finalizes run during
b-tile k+1's first modalities (where DVE is idle), and the last b-tile's
trail after the matmul stream, output muls alternating DVE/Pool. The
missing-modality rescale is applied on the host during the gather (the
host knows the zero rows exactly; straggler zero rows inside the kernel
still contribute exp(0)=1 to Den and 0 to Num naturally, so on-device
detection is unnecessary).

tanh scores lie in [-1,1], so softmax is computed without max-subtraction
as (sum x*exp(tanh s)) / (sum exp(tanh s)).

Missing-modality rows (all-zero x[m, b, :]) contribute exp(0)=1 to the
softmax denominator and 0 to the numerator, so their matmuls are pure
waste. The host detects them, permutes the batch so each core sees the
same per-modality all-zero prefix, and the kernel is built with those
prefixes statically skipped (shorter matmul N, Den bumped by a constant).
The permutation is undone on the host after the gather. With no zero rows
the plan degenerates to a dense kernel.

DMA: wt streams on the ACT ring (idle early), x8 on sync, xe on Pool,
outputs on sync; DRAM layouts are partition-major so every transfer
reads >=2KB contiguous per partition. x tiles are DMA-issued one
work-item ahead; Tile's WAR semaphores start each transfer the moment
its double-buffer slot frees.
"""

from contextlib import ExitStack

import numpy as np
import ml_dtypes

import concourse.bass as bass
import concourse.bacc as bacc
import concourse.mybir as mybir
import concourse.tile as tile
from concourse.bass_utils import run_bass_kernel_spmd

F32 = mybir.dt.float32
FP16 = mybir.dt.float16
FP8 = mybir.dt.float8e4
P = 128
N_CORES = 8
W_SCALE = 16.0  # pow2: exact in fp; folded into tanh via activation scale
GP_DEN_MODS = (1, 2, 3, 4, 5)  # modalities whose den-add runs on Pool


def build_kernel(M, D, E, Bc, BT, skips=None):
    """Build the per-core Bass graph.

    M: modalities, D: contraction dim, E: output feature dim, Bc: per-core
    batch, BT: batch tile (matmul N). skips[m] = per-core all-zero prefix
    length for modality m (columns statically skipped).
    """
    DC = D // P
    D2 = DC // 2  # DoubleRow chunks of K=256
    EC = E // P
    EP = EC // 2  # e-chunk pairs for per-op fusion
    NBT = Bc // BT
    assert D % (2 * P) == 0 and E % (2 * P) == 0 and Bc % BT == 0
    skips = list(skips or [0] * M)
    assert len(skips) == M and all(0 <= k <= Bc for k in skips)

    nc = bacc.Bacc()

    x8 = nc.declare_dram_parameter("x8", [M, NBT, P, DC, BT], FP8, isOutput=False)
    xe = nc.declare_dram_parameter("xe", [M, NBT, P, DC, BT], FP16, isOutput=False)
    wt = nc.declare_dram_parameter("wt", [EC, P, D], FP8, isOutput=False)
    outT = nc.declare_dram_parameter("outT", [E, Bc], F32, isOutput=True)

    with tile.TileContext(nc) as tc, ExitStack() as ctx:
        singles = ctx.enter_context(tc.tile_pool(name="singles", bufs=1))
        xe_pool = ctx.enter_context(tc.tile_pool(name="xe", bufs=2))
        x8_pool = ctx.enter_context(tc.tile_pool(name="x8", bufs=2))
        acc_pool = ctx.enter_context(tc.tile_pool(name="acc", bufs=2))
        e_pool = ctx.enter_context(tc.tile_pool(name="e", bufs=3))
        t_pool = ctx.enter_context(tc.tile_pool(name="t", bufs=2))
        prod_pool = ctx.enter_context(tc.tile_pool(name="prod", bufs=2))
        out_pool = ctx.enter_context(tc.tile_pool(name="out", bufs=3))
        rec_pool = ctx.enter_context(tc.tile_pool(name="rec", bufs=2))
        den_pool = ctx.enter_context(tc.tile_pool(name="den32", bufs=2))
        sc_psum = ctx.enter_context(tc.tile_pool(name="scps", bufs=3, space="PSUM"))
        wu_psum = ctx.enter_context(tc.tile_pool(name="wups", bufs=1, space="PSUM"))

        wu_sb = singles.tile([P, 64], mybir.dt.bfloat16)
        nc.vector.memset(wu_sb, 1.0)
        wu_ps = wu_psum.tile([P, 64], F32)
        for i in range(64):
            nc.tensor.matmul(
                wu_ps[:64], lhsT=wu_sb[:, :64], rhs=wu_sb,
                start=(i == 0), stop=(i == 63),
            )
        wu_out = singles.tile([P, 64], F32)
        nc.vector.tensor_copy(wu_out[:64], wu_ps[:64])

        tiles = {}

        def issue_loads(it):
            bt, m = it
            lo = min(max(skips[m] - bt * BT, 0), BT)
            xe_t = xe_pool.tile([P, DC, BT], FP16)
            x8_t = x8_pool.tile([P, DC, BT], FP8)
            for q in range(0, DC, max(DC // 4, 1)):
                qe = min(q + max(DC // 4, 1), DC)
                nc.sync.dma_start(
                    out=x8_t[:, q:qe, lo:], in_=x8[m, bt, :, q:qe, lo:]
                )
                nc.gpsimd.dma_start(
                    out=xe_t[:, q:qe, lo:], in_=xe[m, bt, :, q:qe, lo:]
                )
            tiles[it] = (xe_t, x8_t)

        def bt_info(bt):
            sk = [min(max(skips[mm] - bt * BT, 0), BT) for mm in range(M)]
            wm = [BT] * (M + 1)
            for mm in range(M):
                wm[mm + 1] = min(wm[mm], sk[mm])
            return sk, wm

        load_seq = []
        for bt in range(NBT):
            sk, _ = bt_info(bt)
            load_seq.extend(
                (bt, mm)
                for mm in sorted(
                    (m for m in range(M) if sk[m] < BT), key=lambda m: sk[m]
                )
            )

        # Replicated fp8 weight, partition-major DRAM layout (2KB contiguous
        # per partition per e-chunk), streamed on the sync ring interleaved
        # with consumption: two e-chunk pairs up front, the rest issued two
        # pairs ahead of the first modality's matmul pairs.
        wt_sb = singles.tile([P, EC, DC, P], FP8)
        wt_loaded = [0]

        def issue_wt_pair(wp):
            if wp < wt_loaded[0] or wp >= EC // 2:
                return
            for ec in (2 * wp, 2 * wp + 1):
                nc.sync.dma_start(out=wt_sb[:, ec, :, :], in_=wt[ec, :, :])
            wt_loaded[0] = wp + 1

        bt0, m0 = load_seq[0]
        lo0 = min(max(skips[m0] - bt0 * BT, 0), BT)
        xe_t = xe_pool.tile([P, DC, BT], FP16)
        x8_t = x8_pool.tile([P, DC, BT], FP8)
        for q in range(0, DC, max(DC // 4, 1)):
            qe = min(q + max(DC // 4, 1), DC)
            nc.sync.dma_start(
                out=wt_sb[:, 0, q:qe, :], in_=wt[0, :, q * P : qe * P]
            )
            nc.sync.dma_start(
                out=x8_t[:, q:qe, lo0:], in_=x8[m0, bt0, :, q:qe, lo0:]
            )
            nc.gpsimd.dma_start(
                out=xe_t[:, q:qe, lo0:], in_=xe[m0, bt0, :, q:qe, lo0:]
            )
        tiles[load_seq[0]] = (xe_t, x8_t)
        nc.sync.dma_start(out=wt_sb[:, 1, :, :], in_=wt[1, :, :])
        wt_loaded[0] = 1
        issue_wt_pair(1)

        loads_done = 1

        def issue_next_load():
            nonlocal loads_done
            if loads_done < len(load_seq):
                issue_loads(load_seq[loads_done])
                loads_done += 1

        # deferred finalizes: list of closures, popped during the next
        # b-tile's early (DVE-idle) windows
        pending_fin = []

        def make_finalize(bt, ep, n_sb, d_sb, sk, w_final):
            def run(split=1, alt=0):
                ec0 = 2 * ep
                ecs = slice(ec0, ec0 + 2)
                # statically-skipped modalities contribute exp(0)=1 to Den
                if w_final > 0:
                    nc.vector.memset(n_sb[:, ecs, :w_final], 0.0)
                    nc.vector.memset(d_sb[:, ecs, :w_final], 0.0)
                for mm in range(M):
                    if sk[mm] > 0:
                        nc.vector.tensor_scalar_add(
                            d_sb[:, ecs, : sk[mm]], d_sb[:, ecs, : sk[mm]], 1.0
                        )
                # fp16 den -> f32 for the bit-trick reciprocal; on the ACT
                # queue, which has headroom
                d32 = den_pool.tile([P, 2, BT], F32, tag="d32")
                nc.scalar.activation(
                    d32, d_sb[:, ecs, :], mybir.ActivationFunctionType.Copy
                )
                H = BT // split
                for h in range(split):
                    hs = slice(h * H, (h + 1) * H)
                    r_t = rec_pool.tile([P, 2, BT], F32, tag="recip")
                    # Den in [1, M*e]: no edge cases; 51-ULP approx is plenty
                    nc.vector.reciprocal_approx_fast(
                        out=r_t[:, :, hs], in_=d32[:, :, hs]
                    )
                    o_t = out_pool.tile([P, 2, BT], F32)
                    mul_eng = nc.gpsimd if alt else nc.vector
                    mul_eng.tensor_mul(
                        o_t[:, :, hs], n_sb[:, ecs, hs], r_t[:, :, hs]
                    )
                    nc.sync.dma_start(
                        out=outT[
                            ec0 * P : (ec0 + 2) * P,
                            bt * BT + h * H : bt * BT + (h + 1) * H,
                        ].rearrange("(c p) b -> p c b", c=2),
                        in_=o_t[:, :, hs],
                    )

            return run

        for bt in range(NBT):
            sk, wm = bt_info(bt)
            w_final = wm[M]
            mods = sorted(
                (mm for mm in range(M) if sk[mm] < BT), key=lambda mm: sk[mm]
            )
            n_sb = acc_pool.tile([P, EC, BT], FP16, tag="num")
            d_sb = acc_pool.tile([P, EC, BT], FP16, tag="den")

            # init watermark in PROCESSING order: columns < watermark are
            # still uninitialized when modality mods[i] runs
            watermark = BT
            for mi, m in enumerate(mods):
                lo = sk[m]  # live range [lo, BT)
                ini_hi = watermark  # init [lo, ini_hi), accum [ini_hi, BT)
                watermark = min(watermark, lo)
                xe_t, x8_t = tiles.pop((bt, m))

                for ep in range(EP):
                    if bt == 0 and mi == 0:
                        issue_wt_pair(ep + 2)
                    sc_ps = sc_psum.tile([P, 2, BT], F32)
                    for j in range(2):
                        ec = 2 * ep + j
                        for d2 in range(D2):
                            nc.tensor.matmul(
                                sc_ps[:, j, lo:],
                                lhsT=wt_sb[:, ec, 2 * d2 : 2 * d2 + 2, :],
                                rhs=x8_t[:, 2 * d2 : 2 * d2 + 2, lo:],
                                start=(d2 == 0),
                                stop=(d2 == D2 - 1),
                                perf_mode=mybir.MatmulPerfMode.DoubleRow,
                            )
                    # pair-fused activations: tanh over both psum banks at once
                    t2 = t_pool.tile([P, 2, BT], FP16, tag="tanh")
                    nc.scalar.activation(
                        t2[:, :, lo:], sc_ps[:, :, lo:],
                        mybir.ActivationFunctionType.Tanh,
                        scale=1.0 / W_SCALE,
                    )
                    ecs = slice(2 * ep, 2 * ep + 2)
                    e2 = None
                    if ini_hi > lo:
                        nc.scalar.activation(
                            d_sb[:, ecs, lo:ini_hi],
                            t2[:, :, lo:ini_hi],
                            mybir.ActivationFunctionType.Exp,
                        )
                    acc_lo = max(lo, ini_hi)
                    if BT > acc_lo:
                        e2 = e_pool.tile([P, 2, BT], FP16, tag="exp")
                        nc.scalar.activation(
                            e2[:, :, acc_lo:],
                            t2[:, :, acc_lo:],
                            mybir.ActivationFunctionType.Exp,
                        )

                    # paired accumulation over both e-chunks at once
                    if ini_hi > lo:
                        ini_eng = nc.gpsimd if ini_hi == BT else nc.vector
                        ini_eng.tensor_mul(
                            n_sb[:, ecs, lo:ini_hi],
                            xe_t[:, ecs, lo:ini_hi],
                            d_sb[:, ecs, lo:ini_hi],
                        )
                    if BT > acc_lo:
                        last_item = bt == NBT - 1 and m == mods[-1]
                        p2 = prod_pool.tile([P, 2, BT], FP16, tag="prod")
                        nc.vector.tensor_mul(
                            p2[:, :, acc_lo:], xe_t[:, ecs, acc_lo:], e2[:, :, acc_lo:]
                        )
                        nc.vector.tensor_add(
                            n_sb[:, ecs, acc_lo:], n_sb[:, ecs, acc_lo:],
                            p2[:, :, acc_lo:],
                        )
                        den_eng = (
                            nc.gpsimd
                            if (m in GP_DEN_MODS and not last_item)
                            else nc.vector
                        )
                        den_eng.tensor_add(
                            d_sb[:, ecs, acc_lo:], d_sb[:, ecs, acc_lo:],
                            e2[:, :, acc_lo:],
                        )

                    # previous b-tile's finalizes run here, where DVE is idle
                    if pending_fin and mi < 3:
                        pending_fin.pop(0)(alt=(mi + ep) % 2)

                # next x tiles issue after this item's wt prefetches on sync
                issue_next_load()

            assert not pending_fin
            for ep in range(EP):
                pending_fin.append(make_finalize(bt, ep, n_sb, d_sb, sk, w_final))

        # last b-tile's finalizes trail the matmul stream (uncontended ops);
        # alternate the output mul between DVE and Pool
        for i, fin in enumerate(pending_fin):
            fin(split=4 if i == len(pending_fin) - 1 else 1, alt=i % 2)
        pending_fin.clear()

    nc.compile()
    return nc


def plan_shards(x):
    """Assign batch rows to cores so every core sees the same per-modality
    all-zero prefix. Returns (perm [n_cores, Bc] of global row ids,
    skips [M] prefix lengths)."""
    M, B, D = x.shape
    Bc = B // N_CORES
    zero = np.all(x == 0.0, axis=2)  # [M, B] truly-all-zero rows
    zcount = zero.sum(axis=0)
    sig = np.zeros(B, dtype=np.int64)
    for m in range(M):
        sig |= zero[m].astype(np.int64) << m
    # cluster by signature with most-zero rows first; round-robin over cores
    order = np.lexsort((sig, -zcount))
    perm = np.stack([order[c::N_CORES] for c in range(N_CORES)])
    skips = []
    for m in range(M):
        k = Bc
        for c in range(N_CORES):
            nz = np.flatnonzero(~zero[m][perm[c]])
            k = min(k, int(nz[0]) if len(nz) else Bc)
        skips.append(k)
    return perm, skips


def prepare_inputs(x, W, perm, BT):
    """Host-side packing: per-core permuted shard, partition-major layouts;
    fp8 copy for the matmul path, fp16 copy for the elementwise path."""
    M, B, D = x.shape
    Bc = B // N_CORES
    DC = D // P
    EC = D // P
    NBT = Bc // BT
    # wt[ec, p, dc, j] = (W.T * S)[dc*P + p, ec*P + j]
    wt = np.ascontiguousarray(
        (W.T * W_SCALE)
        .astype(ml_dtypes.float8_e4m3)
        .reshape(DC, P, EC, P)
        .transpose(2, 1, 0, 3)
    ).reshape(EC, P, DC * P)
    x8b = x.astype(ml_dtypes.float8_e4m3)
    xeb = x.astype(np.float16)
    in_maps = []
    for c in range(N_CORES):
        # [M, NBT, P, DC, BT] with value (m, bt, p, dc, b) =
        #   x[m, perm[bt*BT+b], dc*P+p]
        x8c = np.ascontiguousarray(
            x8b[:, perm[c], :].reshape(M, NBT, BT, DC, P).transpose(0, 1, 4, 3, 2)
        )
        xec = np.ascontiguousarray(
            xeb[:, perm[c], :].reshape(M, NBT, BT, DC, P).transpose(0, 1, 4, 3, 2)
        )
        in_maps.append({"x8": x8c, "xe": xec, "wt": wt})
    return in_maps


_NC_CACHE = {}


def kernel(x, W, _trace=False, **trace_kwargs):
    x = np.asarray(x)
    W = np.asarray(W)
    M, B, D = x.shape
    Bc = B // N_CORES
    BT = 512 if Bc % 512 == 0 else Bc
    perm, skips = plan_shards(x)
    key = (M, B, D, tuple(skips))
    if key not in _NC_CACHE:
        _NC_CACHE[key] = build_kernel(M, D, D, Bc, BT, skips=skips)
    nc = _NC_CACHE[key]
    in_maps = prepare_inputs(x, W, perm, BT)
    res = run_bass_kernel_spmd(
        nc, in_maps, core_ids=list(range(N_CORES)), trace=_trace, **trace_kwargs
    )
    # missing-modality rescale on the host (reference-exact semantics:
    # a modality counts as missing when its feature row sums to 0)
    scaler = ((x.sum(axis=-1) == 0.0).sum(axis=0) + 1).astype(np.float32)
    out = np.empty((B, D), np.float32)
    for c in range(N_CORES):
        out[perm[c], :] = (
            res.results[c]["outT"]
            * scaler[perm[c]][None, :]
        ).T
    if _trace:
        return out, res
    return out
